# revision 28
# baseline (speedup 1.0000x reference)
"""Multi-head attention (nn_Attention_18528488915211) on 8 Trainium2 NeuronCores.

Sharding: tensor-parallel over heads. 16 heads / 8 cores = 2 heads per core.
Each core computes Q/K/V projections for its 256 columns of Wq/Wk/Wv,
attention for its 2 heads, and a partial (transposed) output projection with
its 256 rows of Wo. The host sums the 8 partial outputs (the TP all-reduce),
transposes, and adds bo.

Kernel design (v4):
  - Q/K projections in fp8e4m3 with MatmulPerfMode.DoubleRow (256-deep
    contraction per matmul = 2x PE throughput): weights host-scaled by 64
    (avoids fp8 subnormals), the 1/64^2 folded into the softmax exp scale.
  - All other matmuls bf16 with f32 PSUM accumulate.
  - Attention in 1024-wide query chunks; exp on [128,1024] PSUM tiles;
    softmax row-sum partials accumulated in bf16 on DVE, partition-reduced
    by a ones-matmul, fast reciprocal + per-query scaling (split in halves
    to release the AV accumulator early).
  - Output projection transposed (wo stationary, prefinal^T moving, 1024
    tokens per stationary load); host transposes back.
  - x tiles software-prefetched one chunk ahead (across batch boundaries)
    so the sequential SP DMA queue never gates the PE; wo load deferred
    past the first projection chunk.
"""

import ml_dtypes
import numpy as np

P = 128          # partitions
DM = 2048        # dmodel
DH = 128         # dhead
HPC = 2          # heads per core
DC = HPC * DH    # dmodel columns per core (256)
B = 4            # batch
L = 2048         # sequence length
T = B * L        # total tokens (8192)
KS8 = DM // 256  # fp8 DoubleRow contraction chunks (8)
TC = 512         # token chunk (projection free dim / PSUM bank)
TI = 1024        # attention query chunk
NCORES = 8
SW = 64.0        # fp8 weight scale
SC = 1.0 / (DH * SW * SW)  # exp scale: 1/dhead (applied twice) and 1/SW^2


def _build_nc():
    import concourse.mybir as mybir
    import concourse.tile as tile
    from concourse import bacc

    f32 = mybir.dt.float32
    bf16 = mybir.dt.bfloat16
    f8 = mybir.dt.float8e4
    EXP = mybir.ActivationFunctionType.Exp
    DR = mybir.MatmulPerfMode.DoubleRow

    nc = bacc.Bacc("TRN2", target_bir_lowering=False, debug=False,
                   num_devices=NCORES)

    # x8[c, p, i, t] = x^T[(2c+i)*128 + p, t] in fp8
    x8 = nc.dram_tensor("x8", [KS8, P, 2, T], f8, kind="ExternalInput").ap()
    # xb[c, p, u, t] = x^T[(2c+u)*128 + p, t] in bf16
    xb = nc.dram_tensor("xb", [KS8, P, 2, T], bf16, kind="ExternalInput").ap()
    # w[p, c, i, m] = (64*W)[(2c+i)*128 + p, m] in fp8
    wq = nc.dram_tensor("wq", [P, KS8, 2, DC], f8, kind="ExternalInput").ap()
    wk = nc.dram_tensor("wk", [P, KS8, 2, DC], f8, kind="ExternalInput").ap()
    wv = nc.dram_tensor("wv", [DM, DC], bf16, kind="ExternalInput").ap()
    bq = nc.dram_tensor("bq", [DC], f32, kind="ExternalInput").ap()  # *64
    bk = nc.dram_tensor("bk", [DC], f32, kind="ExternalInput").ap()  # *64
    bv = nc.dram_tensor("bv", [DC], f32, kind="ExternalInput").ap()
    wo = nc.dram_tensor("wo", [DC, DM], bf16, kind="ExternalInput").ap()
    # transposed partial output: out[n, t]
    out = nc.dram_tensor("out", [DM, T], bf16, kind="ExternalOutput").ap()

    with tile.TileContext(nc) as tc:
        with (
            tc.tile_pool(name="wpool", bufs=1) as wpool,
            tc.tile_pool(name="xpool", bufs=16) as xpool,
            tc.tile_pool(name="qkv", bufs=1) as qkv,
            tc.tile_pool(name="ptp", bufs=6) as ptp,
            tc.tile_pool(name="misc", bufs=2) as misc,
            tc.tile_pool(name="ps", bufs=2, space="PSUM") as ps,
        ):
            # --- resident weights/constants ---
            # weights issued on the ACT queue so they don't delay the x
            # tiles behind them on the (serial) SP DMA sequencer
            wq_sb = wpool.tile([P, KS8, 2, DC], f8, tag="wq")
            wk_sb = wpool.tile([P, KS8, 2, DC], f8, tag="wk")
            for c in range(KS8):
                nc.scalar.dma_start(wq_sb[:, c, :, :], wq[:, c, :, :])
                nc.scalar.dma_start(wk_sb[:, c, :, :], wk[:, c, :, :])
            bq_sb = wpool.tile([P, HPC], f32, tag="bq")
            bk_sb = wpool.tile([P, HPC], f32, tag="bk")
            nc.scalar.dma_start(bq_sb[:], bq.rearrange("(h d) -> d h", d=P))
            nc.scalar.dma_start(bk_sb[:], bk.rearrange("(h d) -> d h", d=P))
            bv_sb = wpool.tile([P, 2 * DC], f32, tag="bv")
            for u in range(2):
                nc.scalar.dma_start(bv_sb[:, u * DC:(u + 1) * DC],
                                    bv[None, :].to_broadcast((P, DC)))
            ones_sb = wpool.tile([P, P], bf16, tag="ones")
            nc.any.memset(ones_sb[:], 1.0)
            wv_sb = wpool.tile([P, DM // P, DC], bf16, tag="wv")
            wo_sb = wpool.tile([P, HPC, DM], bf16, tag="wo")

            def fetch_chunk(b, tci):
                ts = b * L + tci * TC
                x8s, xbs = [], []
                for c in range(KS8):
                    x8_t = xpool.tile([P, 2, TC], f8, tag="x8")
                    nc.sync.dma_start(x8_t[:], x8[c, :, :, ts:ts + TC])
                    x8s.append(x8_t)
                for c in range(KS8):
                    xb_t = xpool.tile([P, 2, TC], bf16, tag="xb")
                    nc.sync.dma_start(xb_t[:], xb[c, :, :, ts:ts + TC])
                    xbs.append(xb_t)
                return x8s, xbs

            pending = fetch_chunk(0, 0)
            for ks in range(DM // P):
                nc.scalar.dma_start(wv_sb[:, ks, :],
                                    wv[ks * P:(ks + 1) * P, :])

            # Deferred work: each (h, ic)'s softmax finalize (rowsum reduce,
            # reciprocal, normalize) is emitted a few iterations into the
            # NEXT h-loop so the in-order PE stream never waits on it; each
            # ic's output projection is emitted after the following h-loop
            # (by which point both heads' finalizes have run).
            pending_fin = [None]
            pending_op = [None]

            def emit_oproj(arg):
                t0_, i0_, ot_l = arg
                for nb in range(DM // P):
                    o_ps = ps.tile([P, TI], f32, tag="st", name="o_ps",
                                   bufs=2)
                    for h in range(HPC):
                        for u in range(2):
                            nc.tensor.matmul(
                                o_ps[:, u * TC:(u + 1) * TC],
                                wo_sb[:, h, nb * P:(nb + 1) * P],
                                ot_l[:, h, i0_ + u * TC:i0_ + (u + 1) * TC],
                                start=(h == 0), stop=(h == HPC - 1),
                            )
                    oout = misc.tile([P, TI], bf16, tag="oout",
                                     name="oout", bufs=4)
                    if nb % 3 == 2:
                        nc.scalar.copy(oout[:], o_ps[:])
                    else:
                        nc.vector.tensor_copy(oout[:], o_ps[:])
                    nc.sync.dma_start(
                        out[nb * P:(nb + 1) * P, t0_ + i0_:t0_ + i0_ + TI],
                        oout[:])

            for b in range(B):
                t0 = b * L
                qt_sb = qkv.tile([P, HPC, L], bf16, tag="qt", name="qt",
                                 bufs=2)
                kt_sb = qkv.tile([P, HPC, L], bf16, tag="kt", name="kt",
                                 bufs=2)
                # v[p, jp, u*DC + c] = V[(2*jp+u)*128 + p, c]
                v_sb = qkv.tile([P, L // P // 2, 2 * DC], bf16, tag="v",
                                name="v", bufs=2)
                ot_sb = qkv.tile([P, HPC, L], bf16, tag="ot", name="ot",
                                 bufs=2)

                # ============ Phase A: Q/K/V projections ============
                chunk = {0: pending}
                for tci in range(L // TC):
                    if tci + 1 < L // TC:
                        chunk[tci + 1] = fetch_chunk(b, tci + 1)
                    x8s, xbs = chunk.pop(tci)
                    # Q^T, K^T via fp8 DoubleRow, weights stationary
                    for w_sb, o_sb, b_sb in ((wq_sb, qt_sb, bq_sb),
                                             (wk_sb, kt_sb, bk_sb)):
                        for h in range(HPC):
                            acc = ps.tile([P, TC], f32, tag="pa", name="qk",
                                          bufs=2)
                            for c in range(KS8):
                                nc.tensor.matmul(
                                    acc[:],
                                    w_sb[:, c, :, h * DH:(h + 1) * DH],
                                    x8s[c][:],
                                    start=(c == 0), stop=(c == KS8 - 1),
                                    perf_mode=DR,
                                )
                            nc.vector.tensor_scalar_add(
                                o_sb[:, h, tci * TC:(tci + 1) * TC],
                                acc[:], b_sb[:, h:h + 1],
                            )
                    # V natural, x stationary; two token blocks per PSUM tile
                    for tp in range(TC // P // 2):
                        acc = ps.tile([P, TC], f32, tag="pa", name="vps",
                                      bufs=2)
                        for ti in range(2):
                            tb = 2 * tp + ti
                            for c in range(KS8):
                                for u in range(2):
                                    nc.tensor.matmul(
                                        acc[:, ti * DC:(ti + 1) * DC],
                                        xbs[c][:, u, tb * P:(tb + 1) * P],
                                        wv_sb[:, 2 * c + u, :],
                                        start=(c == 0 and u == 0),
                                        stop=(c == KS8 - 1 and u == 1),
                                    )
                        nc.vector.tensor_add(
                            v_sb[:, tci * 2 + tp, :], acc[:], bv_sb[:],
                        )
                if b == 0:
                    # wo needed only by the first O-projection; load late so
                    # it does not gate the first x tiles on the DMA queues
                    for h in range(HPC):
                        for q in range(DM // TC):
                            nc.scalar.dma_start(
                                wo_sb[:, h, q * TC:(q + 1) * TC],
                                wo[h * P:(h + 1) * P, q * TC:(q + 1) * TC])
                if b + 1 < B:
                    pending = fetch_chunk(b + 1, 0)

                # ===== Phase B: attention (deferred finalize / O-proj) =====
                NJ = L // P
                LAG = 3
                for ic in range(L // TI):
                    i0 = ic * TI
                    for h in range(HPC):
                        ot_ps = ps.tile([P, TI], f32, tag="ot",
                                        name="ot_ps", bufs=1)
                        racc = misc.tile([P, TI], bf16, tag="racc",
                                         name="racc", bufs=2)
                        rac2 = misc.tile([P, TI], bf16, tag="rac2",
                                         name="rac2", bufs=2)
                        pts = {}
                        for jj in range(NJ + LAG):
                            if jj == 2 and pending_fin[0] is not None:
                                pending_fin[0]()
                                pending_fin[0] = None
                            if jj < NJ:
                                j = jj
                                kt_j = kt_sb[:, h, j * P:(j + 1) * P]
                                st2 = ps.tile([P, TI], f32, tag="st",
                                              name="st2", bufs=2)
                                for u in range(2):
                                    nc.tensor.matmul(
                                        st2[:, u * TC:(u + 1) * TC], kt_j,
                                        qt_sb[:, h,
                                              i0 + u * TC:i0 + (u + 1) * TC],
                                        start=True, stop=True,
                                    )
                                pt = ptp.tile([P, TI], bf16, tag="pt",
                                              name="pt")
                                nc.scalar.activation(pt[:], st2[:], EXP,
                                                     scale=SC)
                                pts[j] = pt
                                # rowsum partials: head chain on gpsimd,
                                # tail chain on DVE (never gates finalize)
                                if j == 1:
                                    nc.gpsimd.tensor_add(rac2[:], pts[0][:],
                                                         pt[:])
                                elif 2 <= j <= 9:
                                    nc.gpsimd.tensor_add(rac2[:], rac2[:],
                                                         pt[:])
                                elif j == 11:
                                    nc.vector.tensor_add(racc[:], pts[10][:],
                                                         pt[:])
                                elif j >= 12:
                                    nc.vector.tensor_add(racc[:], racc[:],
                                                         pt[:])
                            if jj >= LAG:
                                ja = jj - LAG
                                v_j = v_sb[:, ja >> 1,
                                           (ja & 1) * DC + h * DH:
                                           (ja & 1) * DC + (h + 1) * DH]
                                pta = pts[ja]
                                for u in range(2):
                                    nc.tensor.matmul(
                                        ot_ps[:, u * TC:(u + 1) * TC],
                                        v_j, pta[:, u * TC:(u + 1) * TC],
                                        start=(ja == 0), stop=(ja == NJ - 1),
                                    )

                        def fin(racc=racc, rac2=rac2, ot_ps=ot_ps, h=h,
                                i0=i0, ot_l=ot_sb):
                            nc.vector.tensor_add(racc[:], racc[:], rac2[:])
                            rs = ps.tile([P, TI], f32, tag="st", name="rs",
                                         bufs=2)
                            rcp = misc.tile([P, TI], f32, tag="rcp",
                                            name="rcp", bufs=2)
                            for u in range(2):
                                us = slice(u * TC, (u + 1) * TC)
                                nc.tensor.matmul(rs[:, us], ones_sb[:],
                                                 racc[:, us],
                                                 start=True, stop=True)
                                nc.vector.reciprocal_approx_fast(rcp[:, us],
                                                                 rs[:, us])
                                nc.vector.tensor_mul(
                                    ot_l[:, h, i0 + u * TC:i0 + (u + 1) * TC],
                                    ot_ps[:, us], rcp[:, us],
                                )
                        pending_fin[0] = fin

                        if pending_op[0] is not None:
                            emit_oproj(pending_op[0])
                            pending_op[0] = None
                        if h == HPC - 1:
                            pending_op[0] = (t0, i0, ot_sb)

            # flush the last finalize + output projection
            if pending_fin[0] is not None:
                pending_fin[0]()
                pending_fin[0] = None
            if pending_op[0] is not None:
                emit_oproj(pending_op[0])
                pending_op[0] = None

    nc.compile()
    return nc


_NC_CACHE = None


def kernel(**inputs: np.ndarray) -> np.ndarray:
    from concourse.bass_utils import run_bass_kernel_spmd

    global _NC_CACHE
    x = np.asarray(inputs["x"], dtype=np.float32)
    Wq, bq = np.asarray(inputs["Wq"]), np.asarray(inputs["bq"])
    Wk, bk = np.asarray(inputs["Wk"]), np.asarray(inputs["bk"])
    Wv, bv = np.asarray(inputs["Wv"]), np.asarray(inputs["bv"])
    Wo, bo = np.asarray(inputs["Wo"]), np.asarray(inputs["bo"])

    f8 = ml_dtypes.float8_e4m3
    bf = ml_dtypes.bfloat16

    xt = np.ascontiguousarray(x.reshape(T, DM).T)          # [DM, T]
    xr = xt.reshape(KS8, 2, P, T).transpose(0, 2, 1, 3)    # [c, p, i, t]
    x8 = np.ascontiguousarray(xr.astype(f8))
    xbv = np.ascontiguousarray(xr.astype(bf))

    def pack_w8(W):  # [DM, DC] -> [P, KS8, 2, DC] fp8, 64-scaled
        Ws = (W * SW).reshape(KS8, 2, P, DC).transpose(2, 0, 1, 3)
        return np.ascontiguousarray(Ws.astype(f8))

    in_maps = []
    for c in range(NCORES):
        sl = slice(c * DC, (c + 1) * DC)
        in_maps.append({
            "x8": x8,
            "xb": xbv,
            "wq": pack_w8(Wq[:, sl]),
            "wk": pack_w8(Wk[:, sl]),
            "wv": np.ascontiguousarray(Wv[:, sl]).astype(bf),
            "bq": np.ascontiguousarray(bq[sl] * SW).astype(np.float32),
            "bk": np.ascontiguousarray(bk[sl] * SW).astype(np.float32),
            "bv": np.ascontiguousarray(bv[sl]).astype(np.float32),
            "wo": np.ascontiguousarray(Wo[sl, :]).astype(bf),
        })

    if _NC_CACHE is None:
        _NC_CACHE = _build_nc()
    res = run_bass_kernel_spmd(_NC_CACHE, in_maps, core_ids=list(range(NCORES)))

    acc = res.results[0]["out"].astype(np.float32)
    for c in range(1, NCORES):
        acc = acc + res.results[c]["out"].astype(np.float32)
    acc = acc.T + bo[None, :].astype(np.float32)
    return np.ascontiguousarray(acc).reshape(B, L, DM)


# revision 31
# speedup vs baseline: 1.1255x; 1.1255x over previous
"""Multi-head attention (nn_Attention_18528488915211) on 8 Trainium2 NeuronCores.

Sharding: tensor-parallel over heads. 16 heads / 8 cores = 2 heads per core.
Each core computes Q/K/V projections for its 256 columns of Wq/Wk/Wv,
attention for its 2 heads, and a partial (transposed) output projection with
its 256 rows of Wo. The host sums the 8 partial outputs (the TP all-reduce),
transposes, and adds bo.

Kernel design (v4):
  - Q/K projections in fp8e4m3 with MatmulPerfMode.DoubleRow (256-deep
    contraction per matmul = 2x PE throughput): weights host-scaled by 64
    (avoids fp8 subnormals), the 1/64^2 folded into the softmax exp scale.
  - All other matmuls bf16 with f32 PSUM accumulate.
  - Attention in 1024-wide query chunks; exp on [128,1024] PSUM tiles;
    softmax row-sum partials accumulated in bf16 on DVE, partition-reduced
    by a ones-matmul, fast reciprocal + per-query scaling (split in halves
    to release the AV accumulator early).
  - Output projection transposed (wo stationary, prefinal^T moving, 1024
    tokens per stationary load); host transposes back.
  - x tiles software-prefetched one chunk ahead (across batch boundaries)
    so the sequential SP DMA queue never gates the PE; wo load deferred
    past the first projection chunk.
"""

import ml_dtypes
import numpy as np

P = 128          # partitions
DM = 2048        # dmodel
DH = 128         # dhead
HPC = 2          # heads per core
DC = HPC * DH    # dmodel columns per core (256)
B = 4            # batch
L = 2048         # sequence length
T = B * L        # total tokens (8192)
KS8 = DM // 256  # fp8 DoubleRow contraction chunks (8)
TC = 512         # token chunk (projection free dim / PSUM bank)
TI = 1024        # attention query chunk
NCORES = 8
SW = 64.0        # fp8 weight scale
SC = 1.0 / (DH * SW * SW)  # exp scale: 1/dhead (applied twice) and 1/SW^2


def _build_nc():
    import concourse.mybir as mybir
    import concourse.tile as tile
    from concourse import bacc

    f32 = mybir.dt.float32
    bf16 = mybir.dt.bfloat16
    f8 = mybir.dt.float8e4
    EXP = mybir.ActivationFunctionType.Exp
    DR = mybir.MatmulPerfMode.DoubleRow

    nc = bacc.Bacc("TRN2", target_bir_lowering=False, debug=False,
                   num_devices=NCORES)

    # x8[c, p, i, t] = x^T[(2c+i)*128 + p, t] in fp8
    x8 = nc.dram_tensor("x8", [KS8, P, 2, T], f8, kind="ExternalInput").ap()
    # xb[c, p, u, t] = x^T[(2c+u)*128 + p, t] in bf16
    xb = nc.dram_tensor("xb", [KS8, P, 2, T], bf16, kind="ExternalInput").ap()
    # w[p, c, i, m] = (64*W)[(2c+i)*128 + p, m] in fp8
    wq = nc.dram_tensor("wq", [P, KS8, 2, DC], f8, kind="ExternalInput").ap()
    wk = nc.dram_tensor("wk", [P, KS8, 2, DC], f8, kind="ExternalInput").ap()
    wv = nc.dram_tensor("wv", [DM, DC], bf16, kind="ExternalInput").ap()
    bq = nc.dram_tensor("bq", [DC], f32, kind="ExternalInput").ap()  # *64
    bk = nc.dram_tensor("bk", [DC], f32, kind="ExternalInput").ap()  # *64
    bv = nc.dram_tensor("bv", [DC], f32, kind="ExternalInput").ap()
    wo = nc.dram_tensor("wo", [DC, DM], bf16, kind="ExternalInput").ap()
    # transposed partial output: out[n, t]
    out = nc.dram_tensor("out", [DM, T], bf16, kind="ExternalOutput").ap()

    with tile.TileContext(nc) as tc:
        with (
            tc.tile_pool(name="wpool", bufs=1) as wpool,
            tc.tile_pool(name="xpool", bufs=16) as xpool,
            tc.tile_pool(name="qkv", bufs=1) as qkv,
            tc.tile_pool(name="ptp", bufs=6) as ptp,
            tc.tile_pool(name="misc", bufs=2) as misc,
            tc.tile_pool(name="ps", bufs=2, space="PSUM") as ps,
        ):
            # --- resident weights/constants ---
            # weights issued on the ACT queue so they don't delay the x
            # tiles behind them on the (serial) SP DMA sequencer
            wq_sb = wpool.tile([P, KS8, 2, DC], f8, tag="wq")
            wk_sb = wpool.tile([P, KS8, 2, DC], f8, tag="wk")
            for c in range(KS8):
                nc.scalar.dma_start(wq_sb[:, c, :, :], wq[:, c, :, :])
                nc.scalar.dma_start(wk_sb[:, c, :, :], wk[:, c, :, :])
            bq_sb = wpool.tile([P, HPC], f32, tag="bq")
            bk_sb = wpool.tile([P, HPC], f32, tag="bk")
            nc.scalar.dma_start(bq_sb[:], bq.rearrange("(h d) -> d h", d=P))
            nc.scalar.dma_start(bk_sb[:], bk.rearrange("(h d) -> d h", d=P))
            bv_sb = wpool.tile([P, 2 * DC], f32, tag="bv")
            for u in range(2):
                nc.scalar.dma_start(bv_sb[:, u * DC:(u + 1) * DC],
                                    bv[None, :].to_broadcast((P, DC)))
            ones_sb = wpool.tile([P, P], bf16, tag="ones")
            nc.any.memset(ones_sb[:], 1.0)
            wv_sb = wpool.tile([P, DM // P, DC], bf16, tag="wv")
            wo_sb = wpool.tile([P, HPC, DM], bf16, tag="wo")

            def fetch_chunk(b, tci):
                ts = b * L + tci * TC
                x8s, xbs = [], []
                for c in range(KS8):
                    x8_t = xpool.tile([P, 2, TC], f8, tag="x8")
                    nc.sync.dma_start(x8_t[:], x8[c, :, :, ts:ts + TC])
                    x8s.append(x8_t)
                for c in range(KS8):
                    xb_t = xpool.tile([P, 2, TC], bf16, tag="xb")
                    nc.sync.dma_start(xb_t[:], xb[c, :, :, ts:ts + TC])
                    xbs.append(xb_t)
                return x8s, xbs

            pending = fetch_chunk(0, 0)
            for ks in range(DM // P):
                nc.scalar.dma_start(wv_sb[:, ks, :],
                                    wv[ks * P:(ks + 1) * P, :])

            # Deferred work: each (h, ic)'s softmax finalize (rowsum reduce,
            # reciprocal, normalize) is emitted a few iterations into the
            # NEXT h-loop so the in-order PE stream never waits on it; each
            # ic's output projection is emitted after the following h-loop
            # (by which point both heads' finalizes have run).
            pending_fin = [None]
            pending_op = [None]

            def emit_oproj(arg):
                t0_, i0_, ot_l = arg
                for nb in range(DM // P):
                    o_ps = ps.tile([P, TI], f32, tag="st", name="o_ps",
                                   bufs=2)
                    for h in range(HPC):
                        for u in range(2):
                            nc.tensor.matmul(
                                o_ps[:, u * TC:(u + 1) * TC],
                                wo_sb[:, h, nb * P:(nb + 1) * P],
                                ot_l[:, h, i0_ + u * TC:i0_ + (u + 1) * TC],
                                start=(h == 0), stop=(h == HPC - 1),
                            )
                    oout = misc.tile([P, TI], bf16, tag="oout",
                                     name="oout", bufs=4)
                    if nb % 3 == 2:
                        nc.scalar.copy(oout[:], o_ps[:])
                    else:
                        nc.vector.tensor_copy(oout[:], o_ps[:])
                    nc.sync.dma_start(
                        out[nb * P:(nb + 1) * P, t0_ + i0_:t0_ + i0_ + TI],
                        oout[:])

            for b in range(B):
                t0 = b * L
                qt_sb = qkv.tile([P, HPC, L], bf16, tag="qt", name="qt",
                                 bufs=2)
                kt_sb = qkv.tile([P, HPC, L], bf16, tag="kt", name="kt",
                                 bufs=2)
                # v[p, jp, u*DC + c] = V[(2*jp+u)*128 + p, c]
                v_sb = qkv.tile([P, L // P // 2, 2 * DC], bf16, tag="v",
                                name="v", bufs=2)
                ot_sb = qkv.tile([P, HPC, L], bf16, tag="ot", name="ot",
                                 bufs=2)

                # ============ Phase A: Q/K/V projections ============
                chunk = {0: pending}
                for tci in range(L // TC):
                    if tci + 1 < L // TC:
                        chunk[tci + 1] = fetch_chunk(b, tci + 1)
                    x8s, xbs = chunk.pop(tci)
                    # Q^T, K^T via fp8 DoubleRow, weights stationary
                    for w_sb, o_sb, b_sb in ((wq_sb, qt_sb, bq_sb),
                                             (wk_sb, kt_sb, bk_sb)):
                        for h in range(HPC):
                            acc = ps.tile([P, TC], f32, tag="pa", name="qk",
                                          bufs=2)
                            for c in range(KS8):
                                nc.tensor.matmul(
                                    acc[:],
                                    w_sb[:, c, :, h * DH:(h + 1) * DH],
                                    x8s[c][:],
                                    start=(c == 0), stop=(c == KS8 - 1),
                                    perf_mode=DR,
                                )
                            nc.vector.tensor_scalar_add(
                                o_sb[:, h, tci * TC:(tci + 1) * TC],
                                acc[:], b_sb[:, h:h + 1],
                            )
                    # V natural, x stationary; two token blocks per PSUM tile
                    for tp in range(TC // P // 2):
                        acc = ps.tile([P, TC], f32, tag="pa", name="vps",
                                      bufs=2)
                        for ti in range(2):
                            tb = 2 * tp + ti
                            for c in range(KS8):
                                for u in range(2):
                                    nc.tensor.matmul(
                                        acc[:, ti * DC:(ti + 1) * DC],
                                        xbs[c][:, u, tb * P:(tb + 1) * P],
                                        wv_sb[:, 2 * c + u, :],
                                        start=(c == 0 and u == 0),
                                        stop=(c == KS8 - 1 and u == 1),
                                    )
                        nc.vector.tensor_add(
                            v_sb[:, tci * 2 + tp, :], acc[:], bv_sb[:],
                        )
                if b == 0:
                    # wo needed only by the first O-projection; load late so
                    # it does not gate the first x tiles on the DMA queues
                    for h in range(HPC):
                        for q in range(DM // TC):
                            nc.scalar.dma_start(
                                wo_sb[:, h, q * TC:(q + 1) * TC],
                                wo[h * P:(h + 1) * P, q * TC:(q + 1) * TC])
                if b + 1 < B:
                    pending = fetch_chunk(b + 1, 0)

                # ===== Phase B: attention (deferred finalize / O-proj) =====
                NJ = L // P
                LAG = 3
                for ic in range(L // TI):
                    i0 = ic * TI
                    for h in range(HPC):
                        ot_ps = ps.tile([P, TI], f32, tag="ot",
                                        name="ot_ps", bufs=1)
                        racc = misc.tile([P, TI], bf16, tag="racc",
                                         name="racc", bufs=2)
                        pts = {}
                        for jj in range(NJ + LAG):
                            if jj == 2 and pending_fin[0] is not None:
                                pending_fin[0]()
                                pending_fin[0] = None
                            if jj < NJ:
                                j = jj
                                kt_j = kt_sb[:, h, j * P:(j + 1) * P]
                                st2 = ps.tile([P, TI], f32, tag="st",
                                              name="st2", bufs=2)
                                for u in range(2):
                                    nc.tensor.matmul(
                                        st2[:, u * TC:(u + 1) * TC], kt_j,
                                        qt_sb[:, h,
                                              i0 + u * TC:i0 + (u + 1) * TC],
                                        start=True, stop=True,
                                    )
                                pt = ptp.tile([P, TI], bf16, tag="pt",
                                              name="pt")
                                nc.scalar.activation(pt[:], st2[:], EXP,
                                                     scale=SC)
                                pts[j] = pt
                                # rowsum partials on DVE (frees PE)
                                if j == 1:
                                    nc.vector.tensor_add(racc[:], pts[0][:],
                                                         pt[:])
                                elif j >= 2:
                                    nc.vector.tensor_add(racc[:], racc[:],
                                                         pt[:])
                            if jj >= LAG:
                                ja = jj - LAG
                                v_j = v_sb[:, ja >> 1,
                                           (ja & 1) * DC + h * DH:
                                           (ja & 1) * DC + (h + 1) * DH]
                                pta = pts[ja]
                                for u in range(2):
                                    nc.tensor.matmul(
                                        ot_ps[:, u * TC:(u + 1) * TC],
                                        v_j, pta[:, u * TC:(u + 1) * TC],
                                        start=(ja == 0), stop=(ja == NJ - 1),
                                    )

                        def fin(racc=racc, ot_ps=ot_ps, h=h,
                                i0=i0, ot_l=ot_sb):
                            rs = ps.tile([P, TI], f32, tag="st", name="rs",
                                         bufs=2)
                            rcp = misc.tile([P, TI], f32, tag="rcp",
                                            name="rcp", bufs=2)
                            for u in range(2):
                                us = slice(u * TC, (u + 1) * TC)
                                nc.tensor.matmul(rs[:, us], ones_sb[:],
                                                 racc[:, us],
                                                 start=True, stop=True)
                                nc.vector.reciprocal_approx_fast(rcp[:, us],
                                                                 rs[:, us])
                                nc.vector.tensor_mul(
                                    ot_l[:, h, i0 + u * TC:i0 + (u + 1) * TC],
                                    ot_ps[:, us], rcp[:, us],
                                )
                        pending_fin[0] = fin

                        if pending_op[0] is not None:
                            emit_oproj(pending_op[0])
                            pending_op[0] = None
                        if h == HPC - 1:
                            pending_op[0] = (t0, i0, ot_sb)

            # flush the last finalize + output projection
            if pending_fin[0] is not None:
                pending_fin[0]()
                pending_fin[0] = None
            if pending_op[0] is not None:
                emit_oproj(pending_op[0])
                pending_op[0] = None

    nc.compile()
    return nc


_NC_CACHE = None


def kernel(**inputs: np.ndarray) -> np.ndarray:
    from concourse.bass_utils import run_bass_kernel_spmd

    global _NC_CACHE
    x = np.asarray(inputs["x"], dtype=np.float32)
    Wq, bq = np.asarray(inputs["Wq"]), np.asarray(inputs["bq"])
    Wk, bk = np.asarray(inputs["Wk"]), np.asarray(inputs["bk"])
    Wv, bv = np.asarray(inputs["Wv"]), np.asarray(inputs["bv"])
    Wo, bo = np.asarray(inputs["Wo"]), np.asarray(inputs["bo"])

    f8 = ml_dtypes.float8_e4m3
    bf = ml_dtypes.bfloat16

    xt = np.ascontiguousarray(x.reshape(T, DM).T)          # [DM, T]
    xr = xt.reshape(KS8, 2, P, T).transpose(0, 2, 1, 3)    # [c, p, i, t]
    x8 = np.ascontiguousarray(xr.astype(f8))
    xbv = np.ascontiguousarray(xr.astype(bf))

    def pack_w8(W):  # [DM, DC] -> [P, KS8, 2, DC] fp8, 64-scaled
        Ws = (W * SW).reshape(KS8, 2, P, DC).transpose(2, 0, 1, 3)
        return np.ascontiguousarray(Ws.astype(f8))

    in_maps = []
    for c in range(NCORES):
        sl = slice(c * DC, (c + 1) * DC)
        in_maps.append({
            "x8": x8,
            "xb": xbv,
            "wq": pack_w8(Wq[:, sl]),
            "wk": pack_w8(Wk[:, sl]),
            "wv": np.ascontiguousarray(Wv[:, sl]).astype(bf),
            "bq": np.ascontiguousarray(bq[sl] * SW).astype(np.float32),
            "bk": np.ascontiguousarray(bk[sl] * SW).astype(np.float32),
            "bv": np.ascontiguousarray(bv[sl]).astype(np.float32),
            "wo": np.ascontiguousarray(Wo[sl, :]).astype(bf),
        })

    if _NC_CACHE is None:
        _NC_CACHE = _build_nc()
    res = run_bass_kernel_spmd(_NC_CACHE, in_maps, core_ids=list(range(NCORES)))

    acc = res.results[0]["out"].astype(np.float32)
    for c in range(1, NCORES):
        acc = acc + res.results[c]["out"].astype(np.float32)
    acc = acc.T + bo[None, :].astype(np.float32)
    return np.ascontiguousarray(acc).reshape(B, L, DM)
